# revision 129
# baseline (speedup 1.0000x reference)
"""Single-head causal self-attention (B=4, T=4096, C=1024, HS=64) on 8 TRN2 cores.

Sharding: core = 2*b + h; the two cores of batch b split the 8 query blocks
(512 rows each) in a load-balanced interleave: h=0 -> blocks {0,3,4,7},
h=1 -> blocks {1,2,5,6} (equal causal-score work: 80 context chunks each).

The SPMD program is identical on every core; per-core differences are pure
data:
  xt  = x[b].T (shared context, global order)
  xtq = x[b, blocks].T (the core's query rows, gathered host-side)
  thr = causal-mask threshold columns (position-aware, per core)
Slot j processes query block g_j against context prefix [0, 128*NCH[j]);
the last 8 context chunks of each slot are masked with data-driven
thresholds against a ramp constant (handles the diagonal, "future" rows
inside the uniform prefix, and fully-masked padding chunks alike).

Dataflow per core (matmul operands bf16, PSUM f32):
  A1:   K^T tile = Wk.T @ xt[:, tile]; V computed directly in natural
        [ctx, h] orientation (stationary = xt 128-ctx subtile, moving =
        Wv chunk), sub-blocks as interleaved accumulation groups in one
        PSUM bank -- no PE transposes, short psA rotation chain.
  A2_j: Q^T_j = (Wq/8).T @ xtq[:, 512j:512j+512]
  S:    context chunks grouped pair/single alternating: S^T matmuls
        ([128,512]) land in a 2-bank pair tile or 1-bank single; one
        Exp per group on the scalar engine amortizes access latency.
        Masks multiply the bf16 E tile on DVE (data-driven 0/1 tiles);
        group formation is mask-aware (pairs mix masked/unmasked chunks,
        singles prefer unmasked) to keep mask-muls off the critical chain.
  O:    orientation-flipped accumulation: stationary = E subtile
        [128ctx, 128q], moving = [V|1] chunk (65 cols) -> out [128q, 65]
        accumulated in a packed PSUM region (16 regions, 7 per bank;
        bank-granular groups: one start / one stop per bank).
  F:    per region: rec = 1/sum, out = O * rec (DVE, bf16 out halves
        the output-DMA bytes), per-slot DMA out (final slot split in two).

PSUM: psA 2 bufs (A-phase + warmup), pair 2 banks + single 1 bank (S),
packed O accumulators 3 banks. Emission follows DMA availability (the
scheduler's priority); tiles 0-1 stream as 256-col halves to cut the
opening latency; dummy warmup matmuls burn the PE p-state ramp; O's lag
their exp by one group so the PE never round-trips on the scalar engine.
"""

import numpy as np
import ml_dtypes

B, T, C, HS = 4, 4096, 1024, 64
QH = T // 2            # queries per core
NSLOT = 4
NCH = [8, 16, 24, 32]  # uniform context chunks (of 128) per slot
BLOCKS = [[0, 3, 4, 7], [1, 2, 5, 6]]  # global 512-blocks per half
CCH = C // 128

_compiled = None


def _build_program():
    import concourse.bass as bass
    import concourse.mybir as mybir
    import concourse.tile as tile
    from concourse import bacc
    from concourse.masks import make_identity
    from contextlib import ExitStack

    f32 = mybir.dt.float32
    bf16 = mybir.dt.bfloat16

    nc = bacc.Bacc("TRN2", target_bir_lowering=False, debug=False, num_devices=8)

    xt_d = nc.dram_tensor("xt", [C, T], bf16, kind="ExternalInput").ap()
    xtq_d = nc.dram_tensor("xtq", [C, QH], bf16, kind="ExternalInput").ap()
    wkv_d = nc.dram_tensor("wkv", [C, 128], bf16, kind="ExternalInput").ap()
    wq_d = nc.dram_tensor("wq", [C, HS], bf16, kind="ExternalInput").ap()
    thr_d = nc.dram_tensor("thr", [128, 32], f32, kind="ExternalInput").ap()
    out_d = nc.dram_tensor("out", [QH, HS], bf16, kind="ExternalOutput").ap()
    out_r = out_d.rearrange("(q p) h -> p q h", p=128)

    with tile.TileContext(nc) as tc, ExitStack() as ctx:
        consts = ctx.enter_context(tc.tile_pool(name="consts", bufs=1))
        epool = ctx.enter_context(tc.tile_pool(name="epool", bufs=1))
        mpool = ctx.enter_context(tc.tile_pool(name="mpool", bufs=4))

        xt = consts.tile([128, CCH, T], bf16)
        xtq = consts.tile([128, CCH, QH], bf16)
        wkv = consts.tile([128, CCH, 128], bf16)
        wq = consts.tile([128, CCH, HS], bf16)
        kT = consts.tile([64, T], bf16)
        qT = consts.tile([64, QH], bf16)
        vp = consts.tile([128, T // 128, HS + 1], bf16)  # [V | ones]
        ramp = consts.tile([128, 512], f32)
        thr = consts.tile([128, 32], f32)
        id_bf = consts.tile([64, 64], bf16)
        ot_all = consts.tile([128, QH // 128, HS], bf16)

        # all DMA on sync/HWDGE: transfers serialize on the shared DMA
        # engines regardless, and this keeps GPSIMD free for mask tiles.
        # First-use order: wq (A2), Wk half (A1's K), q-tile pairs 0 and 1
        # (slots 0+1 go live together), x0/x1 halves, Wv/thr/ramp, rest.
        xtq_r = xtq_d.rearrange("(a p) t -> p a t", p=128)
        xt_r = xt_d.rearrange("(a p) t -> p a t", p=128)
        wkv_r = wkv_d.rearrange("(a p) m -> p a m", p=128)
        # tiles 0-1 split into 256-col halves: cuts the first-exp latency
        # (the opening is DMA-latency-bound); later tiles stay full-width
        HALF = {0, 1}       # x tiles DMA'd as halves
        QHALF = {0, 1}  # q tiles DMA'd as halves
        order = []
        for i in range(4):
            q = ([("q", i, 0), ("q", i, 1)] if i in QHALF
                 else [("q", i, 0)])
            x = ([("x", i, 0), ("x", i, 1)] if i in HALF
                 else [("x", i, 0)])
            order += q + x
        for i in range(4, 8):
            order += [("x", i, 0)]

        def _w(kind, i):
            return 256 if i in (QHALF if kind == "q" else HALF) else 512

        nc.sync.dma_start(out=wq, in_=wq_d.rearrange("(a p) m -> p a m", p=128))
        nc.sync.dma_start(out=wkv, in_=wkv_r)
        for n, (kind, i, hf) in enumerate(order):
            sl = slice(i * 512 + hf * 256, i * 512 + hf * 256 + _w(kind, i))
            src = xtq_r if kind == "q" else xt_r
            dst = xtq if kind == "q" else xt
            nc.sync.dma_start(out=dst[:, :, sl], in_=src[:, :, sl])
            if n == 3:
                nc.sync.dma_start(out=thr, in_=thr_d)
        # ramp = [0..511] per partition, generated on the idle GPSIMD
        # instead of DMA'd (integers < 2^24 are exact in f32)
        nc.gpsimd.iota(ramp, [[1, 512]], base=0, channel_multiplier=0,
                       allow_small_or_imprecise_dtypes=True)
        make_identity(nc, id_bf)
        nc.vector.memset(vp[:, :, HS], 1.0)
        zwarm = consts.tile([64, 512], bf16)
        nc.vector.memset(zwarm, 0.0)

        # causal-mask tiles: each is consumed by exactly one chunk's
        # multiply, so they rotate through a small pool, generated
        # just-in-time on the otherwise-idle GPSIMD (frees ~24KB of SBUF
        # for the exp-output pipeline)
        mkpool = ctx.enter_context(tc.tile_pool(name="mkpool", bufs=6))

        def get_mk(i):
            t = mkpool.tile([128, 512], bf16, tag="mk", name=f"mk_{i}")
            nc.gpsimd.tensor_scalar(
                t, ramp, thr[:, i:i + 1], None,
                op0=mybir.AluOpType.is_ge)
            return t

        # PSUM: psA 2 bufs (A-phase scratch) + psC pair (2 banks) +
        # single (1 bank) + psD 3 banks (16 packed [128,65] O regions).
        # PSUM accumulation groups are bank-granular (2KB zero regions):
        # per bank, exactly one start=True (first-emitted k==0 sub, which
        # lazily zeroes the bank; later first-touches of other regions
        # write rather than accumulate) and one stop=True (last-emitted
        # accumulate into that bank).
        with tc.tile_pool(name="psA", bufs=2, space="PSUM") as psA, \
             tc.tile_pool(name="psC", bufs=1, space="PSUM") as psC, \
             tc.tile_pool(name="psD", bufs=1, space="PSUM") as psD:
            ot = psD.tile([128, 3, 512], f32)

            def oreg(r):
                o = 65 * (r % 7)
                return ot[:, r // 7, o:o + 65]

            # A-phase atoms: one matmul / copy / transpose each, so the
            # plan can interleave them between pairs (emission order is the
            # scheduler's priority; a contiguous 8-matmul block would
            # monopolize the PE and bubble the exp pipeline)
            live = {}

            def emit_atom(atom):
                kind, i, hf = atom[0], atom[1], atom[2]
                w = _w("q" if kind.startswith("a2") else "x", i)
                sl = slice(i * 512 + hf * 256, i * 512 + hf * 256 + w)
                if kind == "a2mm":
                    ci = atom[3]
                    if ci == 0:
                        live["pq", i, hf] = psA.tile(
                            [64, w], f32, tag="pa", name=f"pq_{i}_{hf}")
                    nc.tensor.matmul(live["pq", i, hf], wq[:, ci, :],
                                     xtq[:, ci, sl],
                                     start=(ci == 0), stop=(ci == CCH - 1))
                elif kind == "a2cp":
                    nc.vector.tensor_copy(qT[:, sl], live.pop(("pq", i, hf)))
                elif kind == "a1mm":
                    ci = atom[3]
                    if ci == 0:
                        live["pa", i, hf] = psA.tile(
                            [64, w], f32, tag="pa", name=f"pa_{i}_{hf}")
                    nc.tensor.matmul(live["pa", i, hf], wkv[:, ci, 0:HS],
                                     xt[:, ci, sl],
                                     start=(ci == 0), stop=(ci == CCH - 1))
                elif kind == "a1cp":
                    nc.vector.tensor_copy(kT[:, sl], live.pop(("pa", i, hf)))
                elif kind == "a1pv":
                    # V in natural [ctx, h] orientation, computed directly:
                    # stationary = xt 128-ctx subtile, moving = Wv chunk.
                    # The sub-blocks accumulate as interleaved groups in one
                    # PSUM bank: one start (lazy-zeroes the bank; later
                    # first-touches write), one stop on the last matmul.
                    nsub = w // 128
                    k0 = i * 4 + hf * 2
                    pv = psA.tile([128, nsub, HS], f32, tag="pa",
                                  name=f"pv_{i}_{hf}")
                    for sub in range(nsub):
                        xoff = i * 512 + hf * 256 + sub * 128
                        for ci in range(CCH):
                            nc.tensor.matmul(
                                pv[:, sub, :], xt[:, ci, xoff:xoff + 128],
                                wkv[:, ci, HS:128],
                                start=(sub == 0 and ci == 0),
                                stop=(sub == nsub - 1 and ci == CCH - 1),
                                skip_group_check=True)
                    nc.vector.tensor_copy(vp[:, k0:k0 + nsub, 0:HS], pv)

            def a2_atoms(j, hf):
                return ([("a2mm", j, hf, ci) for ci in range(CCH)]
                        + [("a2cp", j, hf)])

            def _qw(i):
                return _w("q", i)

            def a1_atoms(m, hf):
                return ([("a1mm", m, hf, ci) for ci in range(CCH)]
                        + [("a1cp", m, hf), ("a1pv", m, hf)])

            def emit_S(chunks, p):
                pc = psC.tile([128, len(chunks), 512], f32,
                              tag=f"pc{len(chunks)}", name=f"pc_{p}")
                et = epool.tile([128, len(chunks), 512], bf16,
                                tag=f"et{len(chunks)}",
                                bufs=(27 if len(chunks) == 2 else 25),
                                name=f"et_{p}")
                for h, (j, k) in enumerate(chunks):
                    nc.tensor.matmul(pc[:, h, :], kT[:, k * 128:k * 128 + 128],
                                     qT[:, j * 512:j * 512 + 512],
                                     start=True, stop=True)
                nc.scalar.activation(et, pc, mybir.ActivationFunctionType.Exp)
                for h, (j, k) in enumerate(chunks):
                    m = k - (NCH[j] - 8)
                    if 0 <= m < 8:
                        mkt = get_mk(8 * j + m)
                        eh = et[:, h, :]
                        nc.vector.tensor_mul(eh, eh, mkt)
                return chunks, et

            def emit_O(pair):
                chunks, et = pair
                for h, (j, k) in enumerate(chunks):
                    for qs in range(4):
                        b = (4 * j + qs) // 7
                        sub = et[:, h, qs * 128:qs * 128 + 128]
                        nc.tensor.matmul(
                            oreg(4 * j + qs), sub, vp[:, k, :],
                            start=(k == 0 and not bank_started[b]),
                            stop=(osub_idx[0] == bank_last[b]),
                            skip_group_check=True)
                        bank_started[b] = True
                        osub_idx[0] += 1
                    if k == NCH[j] - 1:
                        last = j == 3
                        for qs in range(4):
                            r = 4 * j + qs
                            rec = mpool.tile([128, 1], f32, tag="rec",
                                             name=f"rec_{r}")
                            nc.vector.reciprocal(rec, oreg(r)[:, HS:HS + 1])
                            # final slot: overlap the two output DMAs
                            nc.vector.tensor_scalar_mul(
                                ot_all[:, r, :], oreg(r)[:, 0:HS], rec)
                            if last and qs % 2 == 1:
                                nc.sync.dma_start(
                                    out=out_r[:, 4 * j + qs - 1:4 * j + qs + 1, :],
                                    in_=ot_all[:, 4 * j + qs - 1:4 * j + qs + 1, :])
                        if not last:
                            nc.sync.dma_start(
                                out=out_r[:, 4 * j:4 * j + 4, :],
                                in_=ot_all[:, 4 * j:4 * j + 4, :])

            # Plan: compute emission in DMA-availability order, pairs
            # formed FIFO; O's lag their exp by one pair so the PE stream
            # never waits on the scalar-engine round-trip. V transposes
            # of tile m go right after the first pair following A1_m (they
            # must precede the first O of level m in PE order).
            plan = []
            pend = []
            gsize = [1]
            hdone = 0
            qdone = []
            for kind, i, hf in order:
                plan += a2_atoms(i, hf) if kind == "q" else a1_atoms(i, hf)
                nk = _w(kind, i) // 128
                if kind == "q":
                    if hf * 256 + _w("q", i) == 512:  # all qT halves in
                        qdone.append(i)
                        pend += [(i, k) for k in range(hdone) if k < NCH[i]]
                else:
                    k0 = 4 * i + hf * 2
                    hdone = k0 + nk
                    pend += [(j, k) for j in qdone
                             for k in range(k0, k0 + nk) if k < NCH[j]]
                def is_masked(c):
                    return 0 <= c[1] - (NCH[c[0]] - 8) < 8
                while len(pend) >= gsize[0]:
                    g = gsize[0]
                    if g == 2:
                        # avoid double-masked pairs when a mixed partner is
                        # nearby (two serial DVE mask-muls lengthen the
                        # exp->O chain)
                        bi = 1
                        for ii in range(1, min(len(pend), 4)):
                            if is_masked(pend[ii]) != is_masked(pend[0]):
                                bi = ii
                                break
                        plan.append(("pair", [pend[0], pend[bi]]))
                        pend = [c for n, c in enumerate(pend)
                                if n not in (0, bi)]
                    else:
                        bi = 0
                        for ii in range(min(len(pend), 4)):
                            if not is_masked(pend[ii]):
                                bi = ii
                                break
                        plan.append(("pair", [pend[bi]]))
                        pend = pend[:bi] + pend[bi + 1:]
                    gsize[0] = 3 - gsize[0]
            if pend:
                plan.append(("pair", pend))

            # per-bank last-accumulate index (for stop flags)
            bank_last = [-1, -1, -1]
            idx = 0
            for item in plan:
                if item[0] == "pair":
                    for (j, k) in item[1]:
                        for qs in range(4):
                            bank_last[(4 * j + qs) // 7] = idx
                            idx += 1

            # PE p-state warmup: a couple of dummy matmuls pre-ramp the
            # tensor engine while the first DMAs are in flight (more would
            # crowd the psA rotation)
            pwarm = psA.tile([64, 512], f32, tag="pa", name="pwarm")
            for _ in range(2):
                nc.tensor.matmul(pwarm, id_bf, zwarm, start=True, stop=True)

            bank_started = [False, False, False]
            osub_idx = [0]
            prev = None
            npair = 0
            for item in plan:
                if item[0] == "pair":
                    pair = emit_S(item[1], npair)
                    npair += 1
                    if prev is not None:
                        emit_O(prev)
                    prev = pair
                else:
                    emit_atom(item)
            if prev is not None:
                emit_O(prev)

    nc.compile()
    return nc


def _prep_inputs(x, Wq, Wk, Wv):
    bf = ml_dtypes.bfloat16
    wkv = np.concatenate([Wk, Wv], axis=1).astype(bf)   # [C, 128]
    wq = (Wq * 0.125).astype(bf)
    p = np.arange(128, dtype=np.float32)
    in_maps = []
    for core in range(8):
        b, h = core // 2, core % 2
        blocks = BLOCKS[h]
        xt = np.ascontiguousarray(x[b].T).astype(bf)
        xtq = np.concatenate(
            [x[b, g * 512:(g + 1) * 512] for g in blocks], axis=0
        ).T.astype(bf)
        thr = np.zeros((128, 32), np.float32)
        for j in range(NSLOT):
            for m in range(8):
                kk = NCH[j] - 8 + m
                thr[:, 8 * j + m] = 128 * kk + p - 512 * blocks[j]
        in_maps.append({
            "xt": np.ascontiguousarray(xt),
            "xtq": np.ascontiguousarray(xtq),
            "wkv": wkv, "wq": wq, "thr": thr,
        })
    return in_maps


def kernel(x, Wq, Wk, Wv):
    from concourse.bass_utils import run_bass_kernel_spmd

    global _compiled
    if _compiled is None:
        _compiled = _build_program()
    nc = _compiled

    in_maps = _prep_inputs(
        np.asarray(x, np.float32), np.asarray(Wq, np.float32),
        np.asarray(Wk, np.float32), np.asarray(Wv, np.float32),
    )
    res = run_bass_kernel_spmd(nc, in_maps, list(range(8)))
    out = np.empty((B, T, HS), np.float32)
    for core in range(8):
        b, h = core // 2, core % 2
        o = res.results[core]["out"]
        for j, g in enumerate(BLOCKS[h]):
            out[b, g * 512:(g + 1) * 512] = o[j * 512:(j + 1) * 512]
    return out


if __name__ == "__main__":
    rng = np.random.default_rng(0)
    x = rng.standard_normal((B, T, C), dtype=np.float32)
    s = 1 / np.sqrt(C)
    Wq = rng.standard_normal((C, HS), dtype=np.float32) * s
    Wk = rng.standard_normal((C, HS), dtype=np.float32) * s
    Wv = rng.standard_normal((C, HS), dtype=np.float32) * s
    o = kernel(x=x, Wq=Wq, Wk=Wk, Wv=Wv)
    print(o.shape, o.dtype, np.abs(o).mean())
